# revision 5
# baseline (speedup 1.0000x reference)
"""Trainium2 kernel for 4096x4096 single-channel 7x7 valid cross-correlation + bias.

Strategy (v3)
-------------
Conv decomposed into 7 banded-Toeplitz matmuls accumulated in PSUM:

    y[r, c] = sum_j sum_i W[i, j] * x[r+i, c+j]

Per strip of 128 input rows ([K=128 partitions, width] SBUF tile), kernel
column j contributes one TensorEngine matmul:
    lhsT = T_j [128, 128] with T_j[k, m] = W[k-m, j]   (stationary, banded)
    rhs  = X[:, j : j+512]                              (free-dim shift)
accumulating 122 valid output rows x 512 output cols in one PSUM bank.

Sharding: columns across 8 cores (512 output cols each + 6-col halo sliced
host-side).  34 row strips per core.

v5 changes (from the v2/v3/v4 neuron-profile traces):
- Stores padded to 128 partitions: the HWDGE splits a DMA across all 16
  SDMA engines only when the SBUF-side partition count is divisible by 8;
  the 122-row stores fell into a 2-engine fallback (~49 GB/s) which made
  output stores the bottleneck (v4: 50us post-compute tail at 111us total).
  Padded [128, 1024] stores hit ~300 GB/s (probe-verified).
- tmat load split so the j=0 Toeplitz block + first x pair land first.
- Output stores moved from gpsimd (software DGE, ~70 GB/s observed, 24us
  post-compute tail) to the sync-engine hardware DGE queue (16 SDMA engines;
  the Activation HWDGE ring only drives 2 SDMA engines, ~51 GB/s measured).
- PSUM groups of 4 strips (was 8): group g and g+1 use disjoint bank sets,
  so next-group matmuls never wait on current-group drains -> no PE idle
  at group boundaries (which also re-triggered the PE p-state ramp).
- gpsimd engine entirely unused; input loads and output stores both on the
  sync hardware queue (stores queue behind loads in FIFO order, which is
  fine: loads are needed first and the ring runs ~290 GB/s).
"""

import os

import numpy as np
import ml_dtypes

import concourse.bass as bass
import concourse.bacc as bacc_mod
import concourse.mybir as mybir
import concourse.tile as tile
from concourse.bass_utils import run_bass_kernel_spmd

H = 4096          # input rows
W = 4096          # input cols
KH = 7            # kernel rows
KW = 7            # kernel cols
OH = H - KH + 1   # 4090 output rows
OW = W - KW + 1   # 4090 output cols
NCORES = 8
CW = 512          # output cols per core
SW = CW + KW - 1  # 518 input cols per shard
STRIP = 122       # output rows per strip (128 input rows -> 122 outputs)
N_STRIPS = -(-OH // STRIP)   # 34
N_PAIRS = -(-N_STRIPS // 2)  # 17
GROUP = 4                    # strips per j-outer group (PSUM bank phasing)

_BF16 = ml_dtypes.bfloat16


def _strip_mk(s: int) -> tuple[int, int]:
    """(valid output rows, input rows) of strip s."""
    m = min(STRIP, OH - s * STRIP)
    return m, m + KH - 1


def _build_program(bias_val: float) -> bass.Bass:
    nc = bacc_mod.Bacc("TRN2", target_bir_lowering=False)

    x_d = nc.dram_tensor("xs", [N_PAIRS, 128, 2 * SW], mybir.dt.bfloat16,
                         kind="ExternalInput")
    t_d = nc.dram_tensor("tmat", [128, KW * 128], mybir.dt.bfloat16,
                         kind="ExternalInput")
    y_d = nc.dram_tensor("y", [N_PAIRS, 128, 2 * CW], mybir.dt.bfloat16,
                         kind="ExternalOutput")

    with tile.TileContext(nc) as tc:
        with (
            tc.tile_pool(name="const", bufs=1) as constp,
            tc.tile_pool(name="xg", bufs=N_PAIRS) as xgp,
            tc.tile_pool(name="yg", bufs=N_PAIRS) as ygp,
            tc.tile_pool(name="ps", bufs=8, space="PSUM") as psp,
        ):
            t_sb = constp.tile([128, KW * 128], mybir.dt.bfloat16)
            nc.sync.dma_start(t_sb[:, :256], t_d[:, :256])

            xg_tiles = []
            for g in range(N_PAIRS):
                xg = xgp.tile([128, 2 * SW], mybir.dt.bfloat16, name="xg", tag="xg")
                xg_tiles.append(xg)
            nc.sync.dma_start(xg_tiles[0][:, :SW], x_d[0, :, :SW])
            nc.sync.dma_start(xg_tiles[0][:, SW:], x_d[0, :, SW:])
            nc.sync.dma_start(xg_tiles[1][:, :], x_d[1, :, :])
            nc.sync.dma_start(t_sb[:, 256:], t_d[:, 256:])
            for g in range(2, N_PAIRS):
                nc.sync.dma_start(xg_tiles[g][:, :], x_d[g, :, :])

            yg_tiles = {}
            bounds = list(range(0, N_STRIPS - 6, GROUP)) + [N_STRIPS - 6]
            for gi, g0 in enumerate(bounds):
                g1 = bounds[gi + 1] if gi + 1 < len(bounds) else N_STRIPS
                strips = list(range(g0, g1))

                ps_tiles = {}
                for s in strips:
                    ps_tiles[s] = psp.tile([128, CW], mybir.dt.float32, name="ps", tag="ps")

                for j in range(KW):
                    for s in strips:
                        m, k = _strip_mk(s)
                        mw = 128 if m == STRIP else m
                        xg = xg_tiles[s // 2]
                        off = (s % 2) * SW
                        nc.tensor.matmul(
                            ps_tiles[s][:mw, :],
                            t_sb[:k, j * 128:j * 128 + mw],
                            xg[:k, off + j:off + j + CW],
                            start=(j == 0),
                            stop=(j == KW - 1),
                        )

                for s in strips:
                    m, _ = _strip_mk(s)
                    g, h = s // 2, s % 2
                    if h == 0:
                        yg = ygp.tile([128, 2 * CW], mybir.dt.bfloat16,
                                      name="yg", tag="yg")
                        yg_tiles[g] = yg
                    else:
                        yg = yg_tiles[g]
                    dst = yg[:m, h * CW:(h + 1) * CW]
                    src = ps_tiles[s][:m, :]
                    if s % 2 == 0:
                        nc.scalar.activation(
                            dst, src, mybir.ActivationFunctionType.Copy,
                            bias=float(bias_val),
                        )
                    else:
                        nc.vector.tensor_scalar_add(dst, src, float(bias_val))
                    if h == 1 or s == N_STRIPS - 1:
                        nc.sync.dma_start(y_d[g, :, :], yg[:, :])

    nc.compile()
    nc.finalize()
    return nc


def _toeplitz(weight: np.ndarray) -> np.ndarray:
    """[128, 7*128] bf16; block j holds T_j[k, m] = W[k-m, j] (band 0<=k-m<7)."""
    t = np.zeros((128, KW * 128), np.float32)
    for j in range(KW):
        for i in range(KH):
            mm = np.arange(0, 128 - i)
            t[mm + i, j * 128 + mm] = weight[i, j]
    return t.astype(_BF16)


def _pack_shard(x_bf: np.ndarray, c0: int) -> np.ndarray:
    """[17, 128, 2*518] bf16: pair 2 strips per partition line."""
    valid = min(SW, W - c0)
    xs = np.zeros((H + 2 * STRIP, SW), _BF16)  # row padding for edge strips
    xs[:H, :valid] = x_bf[:, c0:c0 + valid]
    packed = np.zeros((N_PAIRS, 128, 2 * SW), _BF16)
    for g in range(N_PAIRS):
        packed[g, :, :SW] = xs[2 * g * STRIP: 2 * g * STRIP + 128]
        packed[g, :, SW:] = xs[(2 * g + 1) * STRIP: (2 * g + 1) * STRIP + 128]
    return packed


def _unpack_out(y_packed: np.ndarray) -> np.ndarray:
    """[17, 122, 1024] bf16 -> [4090, 512] f32."""
    out = np.empty((OH, CW), np.float32)
    for s in range(N_STRIPS):
        m, _ = _strip_mk(s)
        g, h = s // 2, s % 2
        out[s * STRIP: s * STRIP + m, :] = \
            y_packed[g, :m, h * CW:(h + 1) * CW].astype(np.float32)
    return out


def kernel(x: np.ndarray, weight: np.ndarray, bias: np.ndarray) -> np.ndarray:
    x = np.asarray(x, dtype=np.float32)
    weight = np.asarray(weight, dtype=np.float32)
    bias = np.asarray(bias, dtype=np.float32)

    tmat = _toeplitz(weight)
    x_bf = x.astype(_BF16)

    in_maps = []
    for c in range(NCORES):
        in_maps.append({"xs": _pack_shard(x_bf, CW * c), "tmat": tmat})

    nc = _build_program(float(bias[0]))

    trace = bool(int(os.environ.get("CONV_KERNEL_TRACE", "0")))
    res = run_bass_kernel_spmd(nc, in_maps, core_ids=list(range(NCORES)),
                               trace=trace)
    if trace:
        kernel.last_exec_time_ns = res.exec_time_ns

    cols = []
    for c in range(NCORES):
        valid_out = min(CW, OW - CW * c)
        cols.append(_unpack_out(np.asarray(res.results[c]["y"]))[:, :valid_out])
    return np.concatenate(cols, axis=1).astype(np.float32)
